# revision 13
# baseline (speedup 1.0000x reference)
"""Trainium2 Bass kernel for CustomPatchEmbedding (ragged patch gather + two projections).

Strategy (data-parallel over batch, 8 cores x 4 images):
  - The host re-lays-out each image into channel-interleaved sliding 16-row
    slab windows:
      T2[b, y, x, c, dy] = img[b, c, y+dy, x]       (y in [0,512), dy in [0,16))
    In this layout one FULL fine patch (16x16x3c) is ONE contiguous 1536B run
    and one coarse 16-row band (64x16x3c) is ONE contiguous 6KB run. SWDGE
    indirect DMA supports exactly one offset/descriptor per dest partition, so
    the whole per-core gather is 12 instructions (8 fine groups + 4 coarse
    bands) of 128 large descriptors each.
  - The feature reorder this induces is static and folded into host-permuted,
    host-preswizzled bf16 weights.
  - Gather offsets are computed on the host from the xy tensors (int32).
  - Images and weights are bf16; PSUM accumulates fp32; output is fp32.
  - TensorE transposes each gathered 128-feature chunk; PSUM->SBUF cast copies
    alternate Vector/Scalar engines; matmuls accumulate in PSUM. The
    transpose/copy/matmul chain is software-pipelined (transposes run LAG
    chunks ahead) so the PE never stalls at its FIFO head waiting for a copy.

kernel(**inputs) takes the FULL unsharded inputs and returns (32, 288, 256) f32.
"""
import sys
import numpy as np

sys.path.insert(0, "/opt/trn_rl_repo")

import ml_dtypes
import concourse.bass as bass
import concourse.bacc as bacc
import concourse.mybir as mybir
import concourse.tile as tile
from concourse.bass_utils import run_bass_kernel_spmd
from contextlib import ExitStack

# Problem constants (hardcoded per spec).
B, C, H, W = 32, 3, 512, 512
FP, CP = 16, 64
NF, NCO = 256, 32
D = 256
NCORES = 8
IPC = B // NCORES              # images per core
KF = C * FP * FP               # 768  fine features
KC = C * CP * CP               # 12288 coarse features
P = 128
NGRP_F = IPC * 2               # 8 fine groups of 128 patches
NKF = KF // P                  # 6 fine k-chunks
NKC = KC // P                  # 96 coarse k-chunks
NBND_C = CP // FP              # 4 coarse bands
BNDC = CP * C * FP             # 3072 elements per coarse band
KPB = BNDC // P                # 24 k-chunks per coarse band
XPITCH = C * FP                # 48 elements per x column in slab layout
SLAB = W * XPITCH              # 24576 elements per slab row
NSLAB = IPC * H * SLAB         # slab tensor elements per core (~50.3M)
LAG = 3                        # transpose->matmul software pipeline depth

FDT = mybir.dt.float32
BDT = mybir.dt.bfloat16
IDT = mybir.dt.int32
BF16 = ml_dtypes.bfloat16


def _emit(nc, tc, t):
    """Emit the per-core Tile program. `t` maps tensor name -> dram handle."""
    with ExitStack() as ctx:
        const = ctx.enter_context(tc.tile_pool(name="const", bufs=1))
        gf_pool = ctx.enter_context(tc.tile_pool(name="gf", bufs=1))
        gc_pool = ctx.enter_context(tc.tile_pool(name="gc", bufs=1))
        lt_pool = ctx.enter_context(tc.tile_pool(name="lt", bufs=2 * LAG + 2))
        ob_pool = ctx.enter_context(tc.tile_pool(name="ob", bufs=3))
        ps_tp = ctx.enter_context(tc.tile_pool(name="ps_tp", bufs=LAG + 2, space="PSUM"))
        ps_f = ctx.enter_context(tc.tile_pool(name="ps_f", bufs=2, space="PSUM"))
        ps_c = ctx.enter_context(tc.tile_pool(name="ps_c", bufs=1, space="PSUM"))

        # --- constants; index tensors go via the scalar HWDGE queue so the
        # gathers can issue as early as possible ---
        fidx = const.tile([P, NGRP_F], IDT)
        nc.scalar.dma_start(fidx[:], t["fidx"][:])
        cidx = const.tile([P, NBND_C], IDT)
        nc.scalar.dma_start(cidx[:], t["cidx"][:])
        identity = const.tile([P, P], BDT)
        nc.sync.dma_start(identity[:], t["ident"][:])
        wf = const.tile([P, NKF * D], BDT)
        nc.sync.dma_start(wf[:], t["wf_sb"][:])
        bias_f = const.tile([P, D], FDT)
        nc.sync.dma_start(bias_f[:], t["bias_f"][:])
        bias_c = const.tile([P, D], FDT)
        nc.sync.dma_start(bias_c[:], t["bias_c"][:])

        slabs = t["slabs"]
        out = t["out"]

        # --- all 12 gathers issued back-to-back into resident tiles ---
        fine_t = []
        for g in range(NGRP_F):
            gt = gf_pool.tile([P, KF], BDT, tag=f"gt{g}")
            nc.gpsimd.indirect_dma_start(
                out=gt[:], out_offset=None, in_=slabs[:],
                in_offset=bass.IndirectOffsetOnAxis(ap=fidx[:, g:g + 1], axis=0),
            )
            fine_t.append(gt)
        coarse_t = []
        for bnd in range(NBND_C):
            ct = gc_pool.tile([P, BNDC], BDT, tag=f"ct{bnd}")
            nc.gpsimd.indirect_dma_start(
                out=ct[:], out_offset=None, in_=slabs[:],
                in_offset=bass.IndirectOffsetOnAxis(ap=cidx[:, bnd:bnd + 1], axis=0),
            )
            coarse_t.append(ct)

        # --- coarse weights: 2 chunked DMAs (second half can land late) ---
        NWCH = 2
        KPW = NKC // NWCH            # 48 k-chunks per weight chunk
        wc_t = []
        for c in range(NWCH):
            wct = const.tile([P, KPW * D], BDT, tag=f"wc{c}")
            nc.sync.dma_start(wct[:], t["wc_sb"][:, c * KPW * D:(c + 1) * KPW * D])
            wc_t.append(wct)

        # Software pipeline: transpose+copy run LAG work-items ahead of the
        # matmul that consumes them, so the PE FIFO never stalls on a copy.
        pend = []
        ncopy = [0]

        def epilogue(psum, bias, rows):
            ob = ob_pool.tile([P, D], FDT, tag="ob")
            nc.vector.tensor_tensor(
                out=ob[:], in0=psum[:], in1=bias[:], op=mybir.AluOpType.add
            )
            for r0, r1, p0 in rows:
                nc.sync.dma_start(out[r0:r1, :], ob[p0:p0 + (r1 - r0), :])

        def push(src, kk, w, wslice, psum, start, stop, on_stop=None):
            tp = ps_tp.tile([P, P], BDT, tag="tp")
            nc.tensor.transpose(
                out=tp[:], in_=src[:, kk * P:(kk + 1) * P], identity=identity[:]
            )
            lt = lt_pool.tile([P, P], BDT, tag="lt")
            if ncopy[0] % 2 == 0:
                nc.vector.tensor_copy(lt[:], tp[:])
            else:
                nc.scalar.copy(lt[:], tp[:])
            ncopy[0] += 1
            pend.append((lt, w, wslice, psum, start, stop, on_stop))
            if len(pend) > LAG:
                fire(1)

        def fire(n):
            for _ in range(n):
                lt, w, wslice, psum, start, stop, on_stop = pend.pop(0)
                nc.tensor.matmul(
                    out=psum[:], lhsT=lt[:],
                    rhs=w[:, wslice * D:(wslice + 1) * D],
                    start=start, stop=stop,
                )
                if on_stop is not None:
                    on_stop()

        # --- fine branch: 8 groups of 128 patches ---
        for g in range(NGRP_F):
            b, h = divmod(g, 2)
            gt = fine_t[g]
            psum = ps_f.tile([P, D], FDT, tag="psf")
            r0 = b * (NF + NCO) + h * P
            ep = (lambda ps, rows: lambda: epilogue(ps, bias_f, rows))(
                psum, [(r0, r0 + P, 0)])
            for k in range(NKF):
                push(gt, k, wf, k, psum, start=(k == 0), stop=(k == NKF - 1),
                     on_stop=ep if k == NKF - 1 else None)

        # --- coarse branch: one group of 128 patches, 96 k-chunks ---
        psum_c = ps_c.tile([P, D], FDT)
        crows = [(b * (NF + NCO) + NF, b * (NF + NCO) + NF + NCO, b * NCO)
                 for b in range(IPC)]
        for bnd in range(NBND_C):
            ct = coarse_t[bnd]
            for kk in range(KPB):
                k = bnd * KPB + kk
                push(ct, kk, wc_t[k // KPW], k % KPW, psum_c,
                     start=(k == 0), stop=(k == NKC - 1),
                     on_stop=(lambda: epilogue(psum_c, bias_c, crows))
                     if k == NKC - 1 else None)
        fire(len(pend))


def build(reps: int = 1):
    nc = bacc.Bacc("TRN2", target_bir_lowering=False, debug=False)
    t = {
        "slabs": nc.dram_tensor("slabs", [NSLAB, 1], BDT, kind="ExternalInput"),
        "fidx": nc.dram_tensor("fidx", [P, NGRP_F], IDT, kind="ExternalInput"),
        "cidx": nc.dram_tensor("cidx", [P, NBND_C], IDT, kind="ExternalInput"),
        "wf_sb": nc.dram_tensor("wf_sb", [P, NKF * D], BDT, kind="ExternalInput"),
        "ident": nc.dram_tensor("ident", [P, P], BDT, kind="ExternalInput"),
        "wc_sb": nc.dram_tensor("wc_sb", [P, NKC * D], BDT, kind="ExternalInput"),
        "bias_f": nc.dram_tensor("bias_f", [P, D], FDT, kind="ExternalInput"),
        "bias_c": nc.dram_tensor("bias_c", [P, D], FDT, kind="ExternalInput"),
        "out": nc.dram_tensor("out", [IPC * (NF + NCO), D], FDT, kind="ExternalOutput"),
    }
    with tile.TileContext(nc) as tc:
        for _ in range(reps):
            _emit(nc, tc, t)
    nc.compile()
    return nc


def host_slabs(images_bf16):
    """images_bf16: (IPC, C, H, W) bf16 -> slab tensor (NSLAB,) bf16.

    T2[b, y, x, c, dy] = img[b, c, y+dy, x]; y >= H-FP rows are zero-padded.
    """
    T = np.zeros((IPC, H, W, C, FP), dtype=BF16)
    sw = np.lib.stride_tricks.sliding_window_view(images_bf16, FP, axis=2)
    # sw[b, c, y, x, dy] = img[b, c, y+dy, x], y in [0, H-FP]
    T[:, :H - FP + 1] = sw.transpose(0, 2, 3, 1, 4)
    return T.reshape(-1)


def host_indices(fine_xy, coarse_xy):
    """Per-core slab-gather offsets: fidx [128, 8], cidx [128, 4] (int32)."""
    # fine: col g; partition p = patch (g%2)*128+p of image g//2
    xy = fine_xy.reshape(NGRP_F, P, 2)                 # (8,128,2)
    b = np.arange(NGRP_F)[:, None] // 2                # (8,1)
    fidx = (b * H + xy[:, :, 1]) * SLAB + xy[:, :, 0] * XPITCH
    fidx = fidx.transpose(1, 0)
    # coarse: col j (16-row band); partition p = (img p//32, patch p%32)
    cxy = coarse_xy.reshape(P, 2)
    bb = np.arange(P) // NCO
    jj = np.arange(NBND_C)[None, :]
    cidx = ((bb[:, None] * H + cxy[:, 1:2] + jj * FP) * SLAB
            + cxy[:, 0:1] * XPITCH)
    return (np.ascontiguousarray(fidx.astype(np.int32)),
            np.ascontiguousarray(cidx.astype(np.int32)))


def host_weights(W_fine, W_coarse):
    """Permute features to slab order and swizzle to SBUF layout, bf16."""
    # fine: k = c*256 + dy*16 + dx  ->  k' = (dx*3 + c)*16 + dy
    wfT = np.asarray(W_fine, np.float32).T.reshape(C, FP, FP, D)      # [c,dy,dx,d]
    wfT = wfT.transpose(2, 0, 1, 3).reshape(KF, D)                    # [dx,c,dy,d]
    # coarse: k = c*4096 + (16j+dy)*64 + dx64 -> k' = ((j*64+dx64)*3 + c)*16 + dy
    wcT = np.asarray(W_coarse, np.float32).T.reshape(C, NBND_C, FP, CP, D)
    wcT = wcT.transpose(1, 3, 0, 2, 4).reshape(KC, D)                 # [j,dx,c,dy,d]
    wf_sb = np.ascontiguousarray(
        wfT.reshape(NKF, P, D).transpose(1, 0, 2).reshape(P, NKF * D).astype(BF16))
    wc_sb = np.ascontiguousarray(
        wcT.reshape(NKC, P, D).transpose(1, 0, 2).reshape(P, NKC * D).astype(BF16))
    return wf_sb, wc_sb


def make_in_maps(images, W_fine, b_fine, W_coarse, b_coarse, fine_xy, coarse_xy):
    images = np.asarray(images, dtype=np.float32).astype(BF16)
    fine_xy = np.asarray(fine_xy, dtype=np.int64)
    coarse_xy = np.asarray(coarse_xy, dtype=np.int64)
    wf_sb, wc_sb = host_weights(W_fine, W_coarse)
    bias_f = np.ascontiguousarray(np.repeat(np.asarray(b_fine, np.float32)[None, :], P, axis=0))
    bias_c = np.ascontiguousarray(np.repeat(np.asarray(b_coarse, np.float32)[None, :], P, axis=0))
    ident = np.eye(P, dtype=BF16)
    in_maps = []
    for cid in range(NCORES):
        sl = slice(cid * IPC, (cid + 1) * IPC)
        fidx, cidx = host_indices(fine_xy[sl], coarse_xy[sl])
        in_maps.append({
            "slabs": host_slabs(images[sl]).reshape(NSLAB, 1),
            "fidx": fidx, "cidx": cidx,
            "wf_sb": wf_sb, "wc_sb": wc_sb, "ident": ident,
            "bias_f": bias_f, "bias_c": bias_c,
        })
    return in_maps


_NC_CACHE = []


def _get_nc():
    if not _NC_CACHE:
        _NC_CACHE.append(build())
    return _NC_CACHE[0]


def run(inputs: dict, trace: bool = False):
    nc = _get_nc()
    in_maps = make_in_maps(**inputs)
    res = run_bass_kernel_spmd(nc, in_maps, list(range(NCORES)), trace=trace)
    outs = [
        np.asarray(res.results[c]["out"]).reshape(IPC, NF + NCO, D)
        for c in range(NCORES)
    ]
    return np.concatenate(outs, axis=0), res


def kernel(**inputs) -> np.ndarray:
    out, _ = run(inputs, trace=False)
    return out


# revision 15
# speedup vs baseline: 1.1497x; 1.1497x over previous
"""Trainium2 Bass kernel for CustomPatchEmbedding (ragged patch gather + two projections).

Strategy (data-parallel over batch, 8 cores x 4 images):
  - The host re-lays-out each image into channel-interleaved sliding 16-row
    slab windows:
      T2[b, y, x, c, dy] = img[b, c, y+dy, x]       (y in [0,512), dy in [0,16))
    In this layout one FULL fine patch (16x16x3c) is ONE contiguous 1536B run
    and one coarse 16-row band (64x16x3c) is ONE contiguous 6KB run. SWDGE
    indirect DMA supports exactly one offset/descriptor per dest partition, so
    the whole per-core gather is 12 instructions (8 fine groups + 4 coarse
    bands) of 128 large descriptors each.
  - The feature reorder this induces is static and folded into host-permuted,
    host-preswizzled bf16 weights.
  - Gather offsets are computed on the host from the xy tensors (int32).
  - Images and weights are bf16; PSUM accumulates fp32; output is fp32.
  - TensorE transposes each gathered 128-feature chunk; PSUM->SBUF cast copies
    alternate Vector/Scalar engines; matmuls accumulate in PSUM. The
    transpose/copy/matmul chain is software-pipelined (transposes run LAG
    chunks ahead) so the PE never stalls at its FIFO head waiting for a copy.

kernel(**inputs) takes the FULL unsharded inputs and returns (32, 288, 256) f32.
"""
import sys
import numpy as np

sys.path.insert(0, "/opt/trn_rl_repo")

import ml_dtypes
import concourse.bass as bass
import concourse.bacc as bacc
import concourse.mybir as mybir
import concourse.tile as tile
from concourse.bass_utils import run_bass_kernel_spmd
from contextlib import ExitStack

# Problem constants (hardcoded per spec).
B, C, H, W = 32, 3, 512, 512
FP, CP = 16, 64
NF, NCO = 256, 32
D = 256
NCORES = 8
IPC = B // NCORES              # images per core
KF = C * FP * FP               # 768  fine features
KC = C * CP * CP               # 12288 coarse features
P = 128
NGRP_F = IPC * 2               # 8 fine groups of 128 patches
NKF = KF // P                  # 6 fine k-chunks
NKC = KC // P                  # 96 coarse k-chunks
NBND_C = CP // FP              # 4 coarse bands
BNDC = CP * C * FP             # 3072 elements per coarse band
KPB = BNDC // P                # 24 k-chunks per coarse band
XPITCH = C * FP                # 48 elements per x column in slab layout
SLAB = W * XPITCH              # 24576 elements per slab row
NSLAB = IPC * H * SLAB         # slab tensor elements per core (~50.3M)
LAG = 3                        # transpose->matmul software pipeline depth

FDT = mybir.dt.float32
BDT = mybir.dt.bfloat16
IDT = mybir.dt.int32
BF16 = ml_dtypes.bfloat16


def _emit(nc, tc, t):
    """Emit the per-core Tile program. `t` maps tensor name -> dram handle."""
    with ExitStack() as ctx:
        const = ctx.enter_context(tc.tile_pool(name="const", bufs=1))
        gf_pool = ctx.enter_context(tc.tile_pool(name="gf", bufs=1))
        gc_pool = ctx.enter_context(tc.tile_pool(name="gc", bufs=1))
        lt_pool = ctx.enter_context(tc.tile_pool(name="lt", bufs=2 * LAG + 2))
        ob_pool = ctx.enter_context(tc.tile_pool(name="ob", bufs=3))
        ps_tp = ctx.enter_context(tc.tile_pool(name="ps_tp", bufs=LAG + 2, space="PSUM"))
        ps_f = ctx.enter_context(tc.tile_pool(name="ps_f", bufs=2, space="PSUM"))
        ps_c = ctx.enter_context(tc.tile_pool(name="ps_c", bufs=1, space="PSUM"))

        # --- constants; index tensors go via the scalar HWDGE queue so the
        # gathers can issue as early as possible ---
        fidx = const.tile([P, NGRP_F], IDT)
        nc.scalar.dma_start(fidx[:], t["fidx"][:])
        cidx = const.tile([P, NBND_C], IDT)
        nc.scalar.dma_start(cidx[:], t["cidx"][:])
        identity = const.tile([P, P], BDT)
        nc.sync.dma_start(identity[:], t["ident"][:])
        wf = const.tile([P, NKF * D], BDT)
        nc.sync.dma_start(wf[:], t["wf_sb"][:])
        bias_f = const.tile([P, D], FDT)
        nc.sync.dma_start(bias_f[:], t["bias_f"][:])
        bias_c = const.tile([P, D], FDT)
        nc.sync.dma_start(bias_c[:], t["bias_c"][:])

        slabs = t["slabs"]
        out = t["out"]

        # --- all 12 gathers issued back-to-back into resident tiles ---
        fine_t = []
        for g in range(NGRP_F):
            gt = gf_pool.tile([P, KF], BDT, tag=f"gt{g}")
            nc.gpsimd.indirect_dma_start(
                out=gt[:], out_offset=None, in_=slabs[:],
                in_offset=bass.IndirectOffsetOnAxis(ap=fidx[:, g:g + 1], axis=0),
            )
            fine_t.append(gt)
        coarse_t = []
        for bnd in range(NBND_C):
            ct = gc_pool.tile([P, BNDC], BDT, tag=f"ct{bnd}")
            nc.gpsimd.indirect_dma_start(
                out=ct[:], out_offset=None, in_=slabs[:],
                in_offset=bass.IndirectOffsetOnAxis(ap=cidx[:, bnd:bnd + 1], axis=0),
            )
            coarse_t.append(ct)

        # --- coarse weights: 4 per-band chunks; the DMAs are emitted behind
        # fine epilogues in the sync FIFO (see below) so their transfers start
        # only after the fine gathers have had the HBM pipe to themselves ---
        NWCH = 4
        KPW = NKC // NWCH            # 24 k-chunks per weight chunk
        wc_t = []
        for c in range(NWCH):
            wct = const.tile([P, KPW * D], BDT, tag=f"wc{c}")
            wc_t.append(wct)
        wc_started = [False] * NWCH

        def start_wc(c):
            if not wc_started[c]:
                wc_started[c] = True
                nc.sync.dma_start(
                    wc_t[c][:], t["wc_sb"][:, c * KPW * D:(c + 1) * KPW * D])

        # Software pipeline: transpose+copy run LAG work-items ahead of the
        # matmul that consumes them, so the PE FIFO never stalls on a copy.
        pend = []
        ncopy = [0]

        def epilogue(psum, bias, rows):
            ob = ob_pool.tile([P, D], FDT, tag="ob")
            nc.vector.tensor_tensor(
                out=ob[:], in0=psum[:], in1=bias[:], op=mybir.AluOpType.add
            )
            for r0, r1, p0 in rows:
                nc.sync.dma_start(out[r0:r1, :], ob[p0:p0 + (r1 - r0), :])

        def push(src, kk, w, wslice, psum, start, stop, on_stop=None):
            tp = ps_tp.tile([P, P], BDT, tag="tp")
            nc.tensor.transpose(
                out=tp[:], in_=src[:, kk * P:(kk + 1) * P], identity=identity[:]
            )
            lt = lt_pool.tile([P, P], BDT, tag="lt")
            if ncopy[0] % 2 == 0:
                nc.vector.tensor_copy(lt[:], tp[:])
            else:
                nc.scalar.copy(lt[:], tp[:])
            ncopy[0] += 1
            pend.append((lt, w, wslice, psum, start, stop, on_stop))
            if len(pend) > LAG:
                fire(1)

        def fire(n):
            for _ in range(n):
                lt, w, wslice, psum, start, stop, on_stop = pend.pop(0)
                nc.tensor.matmul(
                    out=psum[:], lhsT=lt[:],
                    rhs=w[:, wslice * D:(wslice + 1) * D],
                    start=start, stop=stop,
                )
                if on_stop is not None:
                    on_stop()

        # --- fine branch: 8 groups of 128 patches ---
        for g in range(NGRP_F):
            b, h = divmod(g, 2)
            gt = fine_t[g]
            psum = ps_f.tile([P, D], FDT, tag="psf")
            r0 = b * (NF + NCO) + h * P
            wcc = g // 2 if g % 2 == 0 else None
            ep = (lambda ps, rows, wcc=wcc: lambda: (
                epilogue(ps, bias_f, rows),
                start_wc(wcc) if wcc is not None else None))(
                psum, [(r0, r0 + P, 0)])
            for k in range(NKF):
                push(gt, k, wf, k, psum, start=(k == 0), stop=(k == NKF - 1),
                     on_stop=ep if k == NKF - 1 else None)

        # --- coarse branch: one group of 128 patches, 96 k-chunks ---
        for c in range(NWCH):
            start_wc(c)
        psum_c = ps_c.tile([P, D], FDT)
        crows = [(b * (NF + NCO) + NF, b * (NF + NCO) + NF + NCO, b * NCO)
                 for b in range(IPC)]
        for bnd in range(NBND_C):
            ct = coarse_t[bnd]
            for kk in range(KPB):
                k = bnd * KPB + kk
                push(ct, kk, wc_t[k // KPW], k % KPW, psum_c,
                     start=(k == 0), stop=(k == NKC - 1),
                     on_stop=(lambda: epilogue(psum_c, bias_c, crows))
                     if k == NKC - 1 else None)
        fire(len(pend))


def build(reps: int = 1):
    nc = bacc.Bacc("TRN2", target_bir_lowering=False, debug=False)
    t = {
        "slabs": nc.dram_tensor("slabs", [NSLAB, 1], BDT, kind="ExternalInput"),
        "fidx": nc.dram_tensor("fidx", [P, NGRP_F], IDT, kind="ExternalInput"),
        "cidx": nc.dram_tensor("cidx", [P, NBND_C], IDT, kind="ExternalInput"),
        "wf_sb": nc.dram_tensor("wf_sb", [P, NKF * D], BDT, kind="ExternalInput"),
        "ident": nc.dram_tensor("ident", [P, P], BDT, kind="ExternalInput"),
        "wc_sb": nc.dram_tensor("wc_sb", [P, NKC * D], BDT, kind="ExternalInput"),
        "bias_f": nc.dram_tensor("bias_f", [P, D], FDT, kind="ExternalInput"),
        "bias_c": nc.dram_tensor("bias_c", [P, D], FDT, kind="ExternalInput"),
        "out": nc.dram_tensor("out", [IPC * (NF + NCO), D], FDT, kind="ExternalOutput"),
    }
    with tile.TileContext(nc) as tc:
        for _ in range(reps):
            _emit(nc, tc, t)
    nc.compile()
    return nc


def host_slabs(images_bf16):
    """images_bf16: (IPC, C, H, W) bf16 -> slab tensor (NSLAB,) bf16.

    T2[b, y, x, c, dy] = img[b, c, y+dy, x]; y >= H-FP rows are zero-padded.
    """
    T = np.zeros((IPC, H, W, C, FP), dtype=BF16)
    sw = np.lib.stride_tricks.sliding_window_view(images_bf16, FP, axis=2)
    # sw[b, c, y, x, dy] = img[b, c, y+dy, x], y in [0, H-FP]
    T[:, :H - FP + 1] = sw.transpose(0, 2, 3, 1, 4)
    return T.reshape(-1)


def host_indices(fine_xy, coarse_xy):
    """Per-core slab-gather offsets: fidx [128, 8], cidx [128, 4] (int32)."""
    # fine: col g; partition p = patch (g%2)*128+p of image g//2
    xy = fine_xy.reshape(NGRP_F, P, 2)                 # (8,128,2)
    b = np.arange(NGRP_F)[:, None] // 2                # (8,1)
    fidx = (b * H + xy[:, :, 1]) * SLAB + xy[:, :, 0] * XPITCH
    fidx = fidx.transpose(1, 0)
    # coarse: col j (16-row band); partition p = (img p//32, patch p%32)
    cxy = coarse_xy.reshape(P, 2)
    bb = np.arange(P) // NCO
    jj = np.arange(NBND_C)[None, :]
    cidx = ((bb[:, None] * H + cxy[:, 1:2] + jj * FP) * SLAB
            + cxy[:, 0:1] * XPITCH)
    return (np.ascontiguousarray(fidx.astype(np.int32)),
            np.ascontiguousarray(cidx.astype(np.int32)))


def host_weights(W_fine, W_coarse):
    """Permute features to slab order and swizzle to SBUF layout, bf16."""
    # fine: k = c*256 + dy*16 + dx  ->  k' = (dx*3 + c)*16 + dy
    wfT = np.asarray(W_fine, np.float32).T.reshape(C, FP, FP, D)      # [c,dy,dx,d]
    wfT = wfT.transpose(2, 0, 1, 3).reshape(KF, D)                    # [dx,c,dy,d]
    # coarse: k = c*4096 + (16j+dy)*64 + dx64 -> k' = ((j*64+dx64)*3 + c)*16 + dy
    wcT = np.asarray(W_coarse, np.float32).T.reshape(C, NBND_C, FP, CP, D)
    wcT = wcT.transpose(1, 3, 0, 2, 4).reshape(KC, D)                 # [j,dx,c,dy,d]
    wf_sb = np.ascontiguousarray(
        wfT.reshape(NKF, P, D).transpose(1, 0, 2).reshape(P, NKF * D).astype(BF16))
    wc_sb = np.ascontiguousarray(
        wcT.reshape(NKC, P, D).transpose(1, 0, 2).reshape(P, NKC * D).astype(BF16))
    return wf_sb, wc_sb


def make_in_maps(images, W_fine, b_fine, W_coarse, b_coarse, fine_xy, coarse_xy):
    images = np.asarray(images, dtype=np.float32).astype(BF16)
    fine_xy = np.asarray(fine_xy, dtype=np.int64)
    coarse_xy = np.asarray(coarse_xy, dtype=np.int64)
    wf_sb, wc_sb = host_weights(W_fine, W_coarse)
    bias_f = np.ascontiguousarray(np.repeat(np.asarray(b_fine, np.float32)[None, :], P, axis=0))
    bias_c = np.ascontiguousarray(np.repeat(np.asarray(b_coarse, np.float32)[None, :], P, axis=0))
    ident = np.eye(P, dtype=BF16)
    in_maps = []
    for cid in range(NCORES):
        sl = slice(cid * IPC, (cid + 1) * IPC)
        fidx, cidx = host_indices(fine_xy[sl], coarse_xy[sl])
        in_maps.append({
            "slabs": host_slabs(images[sl]).reshape(NSLAB, 1),
            "fidx": fidx, "cidx": cidx,
            "wf_sb": wf_sb, "wc_sb": wc_sb, "ident": ident,
            "bias_f": bias_f, "bias_c": bias_c,
        })
    return in_maps


_NC_CACHE = []


def _get_nc():
    if not _NC_CACHE:
        _NC_CACHE.append(build())
    return _NC_CACHE[0]


def run(inputs: dict, trace: bool = False):
    nc = _get_nc()
    in_maps = make_in_maps(**inputs)
    res = run_bass_kernel_spmd(nc, in_maps, list(range(NCORES)), trace=trace)
    outs = [
        np.asarray(res.results[c]["out"]).reshape(IPC, NF + NCO, D)
        for c in range(NCORES)
    ]
    return np.concatenate(outs, axis=0), res


def kernel(**inputs) -> np.ndarray:
    out, _ = run(inputs, trace=False)
    return out


# revision 16
# speedup vs baseline: 1.1937x; 1.0382x over previous
"""Trainium2 Bass kernel for CustomPatchEmbedding (ragged patch gather + two projections).

Strategy (data-parallel over batch, 8 cores x 4 images):
  - The host re-lays-out each image into channel-interleaved sliding 16-row
    slab windows:
      T2[b, y, x, c, dy] = img[b, c, y+dy, x]       (y in [0,512), dy in [0,16))
    In this layout one FULL fine patch (16x16x3c) is ONE contiguous 1536B run
    and one coarse 16-row band (64x16x3c) is ONE contiguous 6KB run. SWDGE
    indirect DMA supports exactly one offset/descriptor per dest partition, so
    the whole per-core gather is 12 instructions (8 fine groups + 4 coarse
    bands) of 128 large descriptors each.
  - The feature reorder this induces is static and folded into host-permuted,
    host-preswizzled bf16 weights.
  - Gather offsets are computed on the host from the xy tensors (int32).
  - Images and weights are bf16; PSUM accumulates fp32; output is fp32.
  - TensorE transposes each gathered 128-feature chunk; PSUM->SBUF cast copies
    alternate Vector/Scalar engines; matmuls accumulate in PSUM. The
    transpose/copy/matmul chain is software-pipelined (transposes run LAG
    chunks ahead) so the PE never stalls at its FIFO head waiting for a copy.

kernel(**inputs) takes the FULL unsharded inputs and returns (32, 288, 256) f32.
"""
import sys
import numpy as np

sys.path.insert(0, "/opt/trn_rl_repo")

import ml_dtypes
import concourse.bass as bass
import concourse.bacc as bacc
import concourse.mybir as mybir
import concourse.tile as tile
from concourse.bass_utils import run_bass_kernel_spmd
from contextlib import ExitStack

# Problem constants (hardcoded per spec).
B, C, H, W = 32, 3, 512, 512
FP, CP = 16, 64
NF, NCO = 256, 32
D = 256
NCORES = 8
IPC = B // NCORES              # images per core
KF = C * FP * FP               # 768  fine features
KC = C * CP * CP               # 12288 coarse features
P = 128
NGRP_F = IPC * 2               # 8 fine groups of 128 patches
NKF = KF // P                  # 6 fine k-chunks
NKC = KC // P                  # 96 coarse k-chunks
NBND_C = CP // FP              # 4 coarse bands
BNDC = CP * C * FP             # 3072 elements per coarse band
KPB = BNDC // P                # 24 k-chunks per coarse band
XPITCH = C * FP                # 48 elements per x column in slab layout
SLAB = W * XPITCH              # 24576 elements per slab row
NSLAB = IPC * H * SLAB         # slab tensor elements per core (~50.3M)
LAG = 3                        # transpose->matmul software pipeline depth

FDT = mybir.dt.float32
BDT = mybir.dt.bfloat16
IDT = mybir.dt.int32
BF16 = ml_dtypes.bfloat16


def _emit(nc, tc, t):
    """Emit the per-core Tile program. `t` maps tensor name -> dram handle."""
    with ExitStack() as ctx:
        const = ctx.enter_context(tc.tile_pool(name="const", bufs=1))
        gf_pool = ctx.enter_context(tc.tile_pool(name="gf", bufs=1))
        gc_pool = ctx.enter_context(tc.tile_pool(name="gc", bufs=1))
        lt_pool = ctx.enter_context(tc.tile_pool(name="lt", bufs=2 * LAG + 2))
        ob_pool = ctx.enter_context(tc.tile_pool(name="ob", bufs=3))
        ps_tp = ctx.enter_context(tc.tile_pool(name="ps_tp", bufs=LAG + 2, space="PSUM"))
        ps_f = ctx.enter_context(tc.tile_pool(name="ps_f", bufs=2, space="PSUM"))
        ps_c = ctx.enter_context(tc.tile_pool(name="ps_c", bufs=1, space="PSUM"))

        # --- constants; index tensors go via the scalar HWDGE queue so the
        # gathers can issue as early as possible ---
        fidx = const.tile([P, NGRP_F], IDT)
        nc.sync.dma_start(fidx[:], t["fidx"][:])
        cidx = const.tile([P, NBND_C], IDT)
        nc.sync.dma_start(cidx[:], t["cidx"][:])
        identity = const.tile([P, P], BDT)
        nc.scalar.dma_start(identity[:], t["ident"][:])
        wf = const.tile([P, NKF * D], BDT)
        nc.scalar.dma_start(wf[:], t["wf_sb"][:])
        bias_f = const.tile([P, D], FDT)
        nc.scalar.dma_start(bias_f[:], t["bias_f"][:])
        bias_c = const.tile([P, D], FDT)
        nc.scalar.dma_start(bias_c[:], t["bias_c"][:])

        slabs = t["slabs"]
        out = t["out"]

        # --- all 12 gathers issued back-to-back into resident tiles ---
        fine_t = []
        for g in range(NGRP_F):
            gt = gf_pool.tile([P, KF], BDT, tag=f"gt{g}")
            nc.gpsimd.indirect_dma_start(
                out=gt[:], out_offset=None, in_=slabs[:],
                in_offset=bass.IndirectOffsetOnAxis(ap=fidx[:, g:g + 1], axis=0),
            )
            fine_t.append(gt)
        coarse_t = []
        for bnd in range(NBND_C):
            ct = gc_pool.tile([P, BNDC], BDT, tag=f"ct{bnd}")
            nc.gpsimd.indirect_dma_start(
                out=ct[:], out_offset=None, in_=slabs[:],
                in_offset=bass.IndirectOffsetOnAxis(ap=cidx[:, bnd:bnd + 1], axis=0),
            )
            coarse_t.append(ct)

        # --- coarse weights: 4 per-band chunks; the DMAs are emitted behind
        # fine epilogues in the sync FIFO (see below) so their transfers start
        # only after the fine gathers have had the HBM pipe to themselves ---
        NWCH = 4
        KPW = NKC // NWCH            # 24 k-chunks per weight chunk
        wc_t = []
        for c in range(NWCH):
            wct = const.tile([P, KPW * D], BDT, tag=f"wc{c}")
            wc_t.append(wct)
        wc_started = [False] * NWCH

        def start_wc(c):
            if not wc_started[c]:
                wc_started[c] = True
                nc.sync.dma_start(
                    wc_t[c][:], t["wc_sb"][:, c * KPW * D:(c + 1) * KPW * D])

        # Software pipeline: transpose+copy run LAG work-items ahead of the
        # matmul that consumes them, so the PE FIFO never stalls on a copy.
        pend = []
        ncopy = [0]

        def epilogue(psum, bias, rows):
            ob = ob_pool.tile([P, D], FDT, tag="ob")
            nc.vector.tensor_tensor(
                out=ob[:], in0=psum[:], in1=bias[:], op=mybir.AluOpType.add
            )
            if rows == "coarse":
                dst = out.ap().rearrange("(b g) d -> b g d", g=NF + NCO)[:, NF:, :]
                nc.sync.dma_start(dst, ob[:])
            else:
                for r0, r1, p0 in rows:
                    nc.sync.dma_start(out[r0:r1, :], ob[p0:p0 + (r1 - r0), :])

        def push(src, kk, w, wslice, psum, start, stop, on_stop=None):
            tp = ps_tp.tile([P, P], BDT, tag="tp")
            nc.tensor.transpose(
                out=tp[:], in_=src[:, kk * P:(kk + 1) * P], identity=identity[:]
            )
            lt = lt_pool.tile([P, P], BDT, tag="lt")
            if ncopy[0] % 2 == 0:
                nc.vector.tensor_copy(lt[:], tp[:])
            else:
                nc.scalar.copy(lt[:], tp[:])
            ncopy[0] += 1
            pend.append((lt, w, wslice, psum, start, stop, on_stop))
            if len(pend) > LAG:
                fire(1)

        def fire(n):
            for _ in range(n):
                lt, w, wslice, psum, start, stop, on_stop = pend.pop(0)
                nc.tensor.matmul(
                    out=psum[:], lhsT=lt[:],
                    rhs=w[:, wslice * D:(wslice + 1) * D],
                    start=start, stop=stop,
                )
                if on_stop is not None:
                    on_stop()

        # --- fine branch: 8 groups of 128 patches ---
        for g in range(NGRP_F):
            b, h = divmod(g, 2)
            gt = fine_t[g]
            psum = ps_f.tile([P, D], FDT, tag="psf")
            r0 = b * (NF + NCO) + h * P
            wcc = g // 2 if g % 2 == 0 else None
            ep = (lambda ps, rows, wcc=wcc: lambda: (
                epilogue(ps, bias_f, rows),
                start_wc(wcc) if wcc is not None else None))(
                psum, [(r0, r0 + P, 0)])
            for k in range(NKF):
                push(gt, k, wf, k, psum, start=(k == 0), stop=(k == NKF - 1),
                     on_stop=ep if k == NKF - 1 else None)

        # --- coarse branch: one group of 128 patches, 96 k-chunks ---
        for c in range(NWCH):
            start_wc(c)
        psum_c = ps_c.tile([P, D], FDT)
        crows = "coarse"
        for bnd in range(NBND_C):
            ct = coarse_t[bnd]
            for kk in range(KPB):
                k = bnd * KPB + kk
                push(ct, kk, wc_t[k // KPW], k % KPW, psum_c,
                     start=(k == 0), stop=(k == NKC - 1),
                     on_stop=(lambda: epilogue(psum_c, bias_c, crows))
                     if k == NKC - 1 else None)
        fire(len(pend))


def build(reps: int = 1):
    nc = bacc.Bacc("TRN2", target_bir_lowering=False, debug=False)
    t = {
        "slabs": nc.dram_tensor("slabs", [NSLAB, 1], BDT, kind="ExternalInput"),
        "fidx": nc.dram_tensor("fidx", [P, NGRP_F], IDT, kind="ExternalInput"),
        "cidx": nc.dram_tensor("cidx", [P, NBND_C], IDT, kind="ExternalInput"),
        "wf_sb": nc.dram_tensor("wf_sb", [P, NKF * D], BDT, kind="ExternalInput"),
        "ident": nc.dram_tensor("ident", [P, P], BDT, kind="ExternalInput"),
        "wc_sb": nc.dram_tensor("wc_sb", [P, NKC * D], BDT, kind="ExternalInput"),
        "bias_f": nc.dram_tensor("bias_f", [P, D], FDT, kind="ExternalInput"),
        "bias_c": nc.dram_tensor("bias_c", [P, D], FDT, kind="ExternalInput"),
        "out": nc.dram_tensor("out", [IPC * (NF + NCO), D], FDT, kind="ExternalOutput"),
    }
    with tile.TileContext(nc) as tc:
        for _ in range(reps):
            _emit(nc, tc, t)
    nc.compile()
    return nc


def host_slabs(images_bf16):
    """images_bf16: (IPC, C, H, W) bf16 -> slab tensor (NSLAB,) bf16.

    T2[b, y, x, c, dy] = img[b, c, y+dy, x]; y >= H-FP rows are zero-padded.
    """
    T = np.zeros((IPC, H, W, C, FP), dtype=BF16)
    sw = np.lib.stride_tricks.sliding_window_view(images_bf16, FP, axis=2)
    # sw[b, c, y, x, dy] = img[b, c, y+dy, x], y in [0, H-FP]
    T[:, :H - FP + 1] = sw.transpose(0, 2, 3, 1, 4)
    return T.reshape(-1)


def host_indices(fine_xy, coarse_xy):
    """Per-core slab-gather offsets: fidx [128, 8], cidx [128, 4] (int32)."""
    # fine: col g; partition p = patch (g%2)*128+p of image g//2
    xy = fine_xy.reshape(NGRP_F, P, 2)                 # (8,128,2)
    b = np.arange(NGRP_F)[:, None] // 2                # (8,1)
    fidx = (b * H + xy[:, :, 1]) * SLAB + xy[:, :, 0] * XPITCH
    fidx = fidx.transpose(1, 0)
    # coarse: col j (16-row band); partition p = (img p//32, patch p%32)
    cxy = coarse_xy.reshape(P, 2)
    bb = np.arange(P) // NCO
    jj = np.arange(NBND_C)[None, :]
    cidx = ((bb[:, None] * H + cxy[:, 1:2] + jj * FP) * SLAB
            + cxy[:, 0:1] * XPITCH)
    return (np.ascontiguousarray(fidx.astype(np.int32)),
            np.ascontiguousarray(cidx.astype(np.int32)))


def host_weights(W_fine, W_coarse):
    """Permute features to slab order and swizzle to SBUF layout, bf16."""
    # fine: k = c*256 + dy*16 + dx  ->  k' = (dx*3 + c)*16 + dy
    wfT = np.asarray(W_fine, np.float32).T.reshape(C, FP, FP, D)      # [c,dy,dx,d]
    wfT = wfT.transpose(2, 0, 1, 3).reshape(KF, D)                    # [dx,c,dy,d]
    # coarse: k = c*4096 + (16j+dy)*64 + dx64 -> k' = ((j*64+dx64)*3 + c)*16 + dy
    wcT = np.asarray(W_coarse, np.float32).T.reshape(C, NBND_C, FP, CP, D)
    wcT = wcT.transpose(1, 3, 0, 2, 4).reshape(KC, D)                 # [j,dx,c,dy,d]
    wf_sb = np.ascontiguousarray(
        wfT.reshape(NKF, P, D).transpose(1, 0, 2).reshape(P, NKF * D).astype(BF16))
    wc_sb = np.ascontiguousarray(
        wcT.reshape(NKC, P, D).transpose(1, 0, 2).reshape(P, NKC * D).astype(BF16))
    return wf_sb, wc_sb


def make_in_maps(images, W_fine, b_fine, W_coarse, b_coarse, fine_xy, coarse_xy):
    images = np.asarray(images, dtype=np.float32).astype(BF16)
    fine_xy = np.asarray(fine_xy, dtype=np.int64)
    coarse_xy = np.asarray(coarse_xy, dtype=np.int64)
    wf_sb, wc_sb = host_weights(W_fine, W_coarse)
    bias_f = np.ascontiguousarray(np.repeat(np.asarray(b_fine, np.float32)[None, :], P, axis=0))
    bias_c = np.ascontiguousarray(np.repeat(np.asarray(b_coarse, np.float32)[None, :], P, axis=0))
    ident = np.eye(P, dtype=BF16)
    in_maps = []
    for cid in range(NCORES):
        sl = slice(cid * IPC, (cid + 1) * IPC)
        fidx, cidx = host_indices(fine_xy[sl], coarse_xy[sl])
        in_maps.append({
            "slabs": host_slabs(images[sl]).reshape(NSLAB, 1),
            "fidx": fidx, "cidx": cidx,
            "wf_sb": wf_sb, "wc_sb": wc_sb, "ident": ident,
            "bias_f": bias_f, "bias_c": bias_c,
        })
    return in_maps


_NC_CACHE = []


def _get_nc():
    if not _NC_CACHE:
        _NC_CACHE.append(build())
    return _NC_CACHE[0]


def run(inputs: dict, trace: bool = False):
    nc = _get_nc()
    in_maps = make_in_maps(**inputs)
    res = run_bass_kernel_spmd(nc, in_maps, list(range(NCORES)), trace=trace)
    outs = [
        np.asarray(res.results[c]["out"]).reshape(IPC, NF + NCO, D)
        for c in range(NCORES)
    ]
    return np.concatenate(outs, axis=0), res


def kernel(**inputs) -> np.ndarray:
    out, _ = run(inputs, trace=False)
    return out
